# revision 24
# baseline (speedup 1.0000x reference)
"""Trainium2 Bass kernel: GPT-2 style causal attention + output projection.

Reference computation (B=2, L=2048, D=1024, H=16, dh=64):
    q,k,v = split_heads(query/key/value)            # [B,H,L,dh]
    S = q @ k^T / sqrt(dh)                          # [B,H,L,L]
    P = softmax(causal_mask(S))
    A = merge_heads(P @ v)                          # [B,L,D]
    out = A @ w_proj + b_proj

Sharding: 32 (b,h) pairs, 4 per core (cores 0-3 batch 0, 4-7 batch 1).
Each core computes attention for its 4 heads fully causally and a partial
c_proj using its 256 rows of w_proj; the host sums the 4 partials per batch
and adds the bias.

Device-side layout: scores are computed transposed (S^T, keys on partitions)
so softmax's P lands with keys on the partition axis, which is exactly the
contraction layout P.V needs.  A ones-column appended to V makes the same
matmul emit softmax denominators.  Two heads ride each 128-partition tile
(dh=64), and their QK matmuls occupy disjoint PE row groups (tile_position)
so they stream concurrently.

The scalar (ACT) engine's exp stream is the pacer (~1us per 128x1024 chunk).
Everything else is scheduled around keeping it and the PE saturated:
  - diagonal-band chunks do column-ranged QK/exp/PV (skips fully-masked
    queries), with the causal triangle applied as a 128-col band multiply
    on GpSimd (the only [128,128] pattern repeats for every band).
  - softmax normalization broadcasts 1/den via GpSimd partition_broadcast
    instead of PE matmuls, freeing the tensor engine.
  - chunks are processed in pairs (QK,QK,...,PV,PV) so PE weight loads
    overlap streaming, and PVs trail QKs by 4 chunks.
  - each q-block's c_proj (4 row-tiles x full 1024 nf) and normalize chains
    are deferred into the next block's chunk stream.
  - q-blocks run largest-first (J=3..0) so the tail is the smallest block.
Partials DMA out as fp16 (halves output traffic; host sums in fp32).
"""

import numpy as np

B, L, D, H = 2, 2048, 1024, 16
DH = 64          # head dim
PAIRS = 4        # (b,h) pairs per core
QB = 512         # query block
KC = 128         # key chunk
NCORES = 8

_COMPILED = None


def _build_nc():
    import concourse.bacc as bacc
    import concourse.tile as tile
    from concourse import mybir

    f32 = mybir.dt.float32
    f16 = mybir.dt.float16
    Exp = mybir.ActivationFunctionType.Exp

    nc = bacc.Bacc("TRN2", target_bir_lowering=False, debug=False,
                   num_devices=NCORES)

    qt_d = nc.dram_tensor("qt", [2, 128, L], f16, kind="ExternalInput").ap()
    kt_d = nc.dram_tensor("kt", [2, 128, L], f16, kind="ExternalInput").ap()
    v_d = nc.dram_tensor("v", [PAIRS, 128, (L // KC) * 128], f16,
                         kind="ExternalInput").ap()
    masks_d = nc.dram_tensor("masks", [128, 256], f16, kind="ExternalInput").ap()
    w_d = nc.dram_tensor("w", [2, 128, D], f16, kind="ExternalInput").ap()
    ones_d = nc.dram_tensor("ones", [1, 64], f16, kind="ExternalInput").ap()
    out_d = nc.dram_tensor("out", [L, D], f16, kind="ExternalOutput").ap()

    with tile.TileContext(nc) as tc:
        with (
            tc.tile_pool(name="consts", bufs=1) as consts,
            tc.tile_pool(name="st", bufs=3, space="PSUM") as st_pool,
            tc.tile_pool(name="at", bufs=2, space="PSUM") as at_pool,
            tc.tile_pool(name="et", bufs=8) as et_pool,
            tc.tile_pool(name="atn", bufs=6) as atn_pool,
            tc.tile_pool(name="dsb", bufs=4) as dsb_pool,
            tc.tile_pool(name="rbc", bufs=3) as rbc_pool,
            tc.tile_pool(name="osb", bufs=4) as osb_pool,
        ):
            # resident inputs
            qt = [consts.tile([128, L], f16, name=f"qt{i}", tag=f"qt{i}")
                  for i in range(2)]
            kt = [consts.tile([128, L], f16, name=f"kt{i}", tag=f"kt{i}")
                  for i in range(2)]
            vt = [consts.tile([128, (L // KC) * 128], f16,
                              name=f"vt{i}", tag=f"vt{i}") for i in range(PAIRS)]
            mk = consts.tile([128, 256], f16, name="mk", tag="mk")
            wt = [consts.tile([128, D], f16, name=f"wt{i}", tag=f"wt{i}")
                  for i in range(2)]
            ones = consts.tile([1, 64], f16, name="ones", tag="ones")

            # DMA order follows the block schedule (J=1 first, then 3, 2, 0):
            # first QK needs kt[0][:,0:256] and qt[0][:,512:1024]; PVs need
            # vt[0]/vt[1] a few chunks later; J=3's tails come next.
            nc.sync.dma_start(ones[:], ones_d[:])
            nc.sync.dma_start(kt[0][:, 0:256], kt_d[0][:, 0:256])
            nc.sync.dma_start(qt[0][:, 512:1024], qt_d[0][:, 512:1024])
            nc.sync.dma_start(kt[0][:, 256:1024], kt_d[0][:, 256:1024])
            nc.sync.dma_start(mk[:], masks_d[:])
            nc.sync.dma_start(vt[0][:], v_d[0])
            nc.sync.dma_start(vt[1][:], v_d[1])
            nc.sync.dma_start(qt[1][:, 512:1024], qt_d[1][:, 512:1024])
            nc.sync.dma_start(kt[1][:, 0:1024], kt_d[1][:, 0:1024])
            nc.sync.dma_start(vt[2][:], v_d[2])
            nc.sync.dma_start(vt[3][:], v_d[3])
            nc.sync.dma_start(kt[0][:, 1024:2048], kt_d[0][:, 1024:2048])
            nc.sync.dma_start(qt[0][:, 1536:2048], qt_d[0][:, 1536:2048])
            nc.sync.dma_start(kt[1][:, 1024:2048], kt_d[1][:, 1024:2048])
            nc.sync.dma_start(qt[1][:, 1536:2048], qt_d[1][:, 1536:2048])
            for i in range(2):
                nc.sync.dma_start(wt[i][:], w_d[i])
            nc.sync.dma_start(qt[0][:, 1024:1536], qt_d[0][:, 1024:1536])
            nc.sync.dma_start(qt[1][:, 1024:1536], qt_d[1][:, 1024:1536])
            nc.sync.dma_start(qt[0][:, 0:512], qt_d[0][:, 0:512])
            nc.sync.dma_start(qt[1][:, 0:512], qt_d[1][:, 0:512])

            # PE p-state warm-up: ~4us of continuous dummy matmuls during the
            # DMA prologue so the first real QKs run at full clock.  Only
            # needs `ones` (the first DMA); the slot is recycled afterwards.
            wu = st_pool.tile([64, 256], f32, name="wu", tag="st")
            for _ in range(14):
                nc.tensor.matmul(wu[:], lhsT=ones[:],
                                 rhs=kt[0][0:1, 0:256],
                                 start=True, stop=True)

            TAIL = [False]   # after the exp stream ends, ACT is free for obs

            def cproj_groups(J, atn_duo):
                def one(rt):
                    def emit():
                        cp = st_pool.tile([128, D], f32, name="cp", tag="st")
                        # duo-major: consecutive matmuls share lhsT weights
                        for duo in range(2):
                            for nf in range(2):
                                nc.tensor.matmul(
                                    cp[:, nf * 512:(nf + 1) * 512],
                                    lhsT=atn_duo[duo][:, rt * 128:(rt + 1) * 128],
                                    rhs=wt[duo][:, nf * 512:(nf + 1) * 512],
                                    start=(duo == 0), stop=(duo == 1),
                                )
                        ob = osb_pool.tile([128, D], f16, name="ob", tag="ob")
                        if TAIL[0] and rt % 2:
                            nc.scalar.copy(ob[:], cp[:])
                        else:
                            nc.vector.tensor_copy(ob[:], cp[:])
                        nc.sync.dma_start(
                            out_d[J * QB + rt * 128:J * QB + (rt + 1) * 128, :],
                            ob[:],
                        )
                    return emit
                return [one(rt) for rt in range(QB // 128)]

            pending = []             # c_proj groups from the previous q-block
            pending_norm = []        # softmax-normalize chains
            pending_pv = []          # tail PVs of the previous duo
            for J in (1, 3, 2, 0):   # largest blocks mid-run, smallest last
                nch = 4 * J + 4      # causal: key chunks 0..nch-1
                atn_duo = []
                for duo in range(2):
                    at = [at_pool.tile([128, QB], f32, name="at", tag="at")
                          for _ in range(2)]
                    ets = {}

                    def emit_qk(c):
                        m = c - 4 * J   # diag band index (>=0 on the band)
                        w0 = max(m, 0) * KC   # first causally-live query col
                        st = st_pool.tile([128, 2 * QB], f32, name="st",
                                          tag="st")
                        for h2 in range(2):
                            nc.tensor.matmul(
                                st[:, h2 * QB + w0:(h2 + 1) * QB],
                                lhsT=kt[duo][64 * h2:64 * (h2 + 1),
                                             c * KC:(c + 1) * KC],
                                rhs=qt[duo][64 * h2:64 * (h2 + 1),
                                            J * QB + w0:(J + 1) * QB],
                                start=True, stop=True,
                                tile_position=(64 * h2, 0),
                            )
                        et = et_pool.tile([128, 2 * QB], f16, name="et",
                                          tag="et")
                        if w0:
                            nc.scalar.activation(
                                et[:].rearrange("p (h q) -> p h q", h=2)
                                [:, :, w0:],
                                st[:].rearrange("p (h q) -> p h q", h=2)
                                [:, :, w0:],
                                Exp, scale=0.125)
                        else:
                            nc.scalar.activation(et[:], st[:], Exp, scale=0.125)
                        if m >= 0:
                            # causal triangle on the diagonal 128-col band;
                            # columns beyond the band are fully visible, and
                            # columns before it are skipped by the ranged
                            # QK/exp/PV
                            ev = et[:].rearrange("p (h q) -> p h q", h=2)[
                                :, :, w0:w0 + KC]
                            mv = mk[:].rearrange("p (h q) -> p h q", h=2)
                            nc.gpsimd.tensor_mul(ev, ev, mv)
                        ets[c] = et

                    def emit_pv(c, ets=ets, at=at, duo=duo, J=J, nch=nch):
                        m = c - 4 * J
                        w0 = max(m, 0) * KC
                        et = ets.pop(c)
                        for h2 in range(2):
                            pair = 2 * duo + h2
                            nc.tensor.matmul(
                                at[h2][0:128, w0:QB],
                                lhsT=vt[pair][:, c * 128:(c + 1) * 128],
                                rhs=et[:, h2 * QB + w0:(h2 + 1) * QB],
                                start=(c == 0), stop=(c == nch - 1),
                                skip_group_check=True,
                            )

                    def pops():
                        # previous-duo PVs and normalize chains are latency
                        # critical (at-slot reuse): drain 2 per iteration.
                        # The c_proj backlog drains at most one group per
                        # iteration, and only once pv/norm are clear, so PE
                        # load stays spread instead of bunching at block
                        # boundaries.  The last q-block (J=0, tiny exp
                        # stream) drains everything: its extra PE work just
                        # shortens the serial tail one-for-one.
                        k = 8 if J == 0 else 2
                        while k and pending_pv:
                            pending_pv.pop(0)()
                            k -= 1
                        while k and pending_norm:
                            pending_norm.pop(0)()
                            k -= 1
                        while k and not pending_pv and not pending_norm \
                                and pending:
                            pending.pop(0)()
                            k -= 1 if J == 0 else k

                    for i in range(nch // 2):
                        emit_qk(2 * i)
                        emit_qk(2 * i + 1)
                        pops()
                        if i >= 2:
                            emit_pv(2 * i - 4)
                            emit_pv(2 * i - 3)
                    # defer the last 4 PVs into the next duo's stream so the
                    # next QKs (and their exps) issue without waiting on this
                    # duo's exp->mask->PV chain
                    pending_pv.extend(
                        [lambda c=c, f=emit_pv: f(c)
                         for c in range(max(nch - 4, 0), nch)])

                    atn = atn_pool.tile([128, QB], f16, name="atn", tag="atn")

                    def norm(at=at, atn=atn):
                        def emit():
                            # fp16 dsb/ones keep the PE in fp16 mode (an fp32
                            # matmul halves the following matmuls' row rate)
                            dsb = dsb_pool.tile([1, 2 * QB], f16, name="dsb",
                                                tag="dsb")
                            for h2 in range(2):
                                nc.vector.tensor_copy(
                                    dsb[:, h2 * QB:(h2 + 1) * QB],
                                    at[h2][64:65, :])
                            bc = st_pool.tile([64, 2 * QB], f32, name="bc",
                                              tag="st")
                            for h2 in range(2):
                                nc.tensor.matmul(
                                    bc[:, h2 * QB:(h2 + 1) * QB],
                                    lhsT=ones[:],
                                    rhs=dsb[:, h2 * QB:(h2 + 1) * QB],
                                    start=True, stop=True)
                            rbc = rbc_pool.tile([64, 2 * QB], f32, name="rbc",
                                                tag="rbc")
                            nc.vector.reciprocal_approx_fast(rbc[:], bc[:])
                            for h2 in range(2):
                                nc.vector.tensor_mul(
                                    atn[64 * h2:64 * (h2 + 1), :],
                                    at[h2][0:64, :],
                                    rbc[:, h2 * QB:(h2 + 1) * QB])
                        return emit

                    pending_norm.append(norm())
                    atn_duo.append(atn)

                if J == 0:
                    leftovers = list(pending)   # previous blocks' remainder
                    pending = []
                    final_groups = cproj_groups(0, atn_duo)
                else:
                    # undrained groups stay queued; pops drain them next block
                    pending.extend(cproj_groups(J, atn_duo))
            # tail: all norms except the last depend only on already-emitted
            # PVs; emit them first so the last duo's PVs (which WAR-wait on
            # the previous duo's normalize reads) don't stall the PE queue
            TAIL[0] = True
            for g in pending_norm[:-1]:
                g()
            for g in pending_pv:
                g()
            pending_norm[-1]()
            for g in leftovers:
                g()
            for g in final_groups:
                g()

    nc.compile()
    return nc


def _get_nc():
    global _COMPILED
    if _COMPILED is None:
        _COMPILED = _build_nc()
    return _COMPILED


def _prep_in_maps(query, key, value, w_proj):
    q = np.asarray(query, dtype=np.float32)
    k = np.asarray(key, dtype=np.float32)
    v = np.asarray(value, dtype=np.float32)
    w = np.asarray(w_proj, dtype=np.float32)

    q4 = q.reshape(B, L, H, DH)
    k4 = k.reshape(B, L, H, DH)
    v4 = v.reshape(B, L, H, DH)

    # causal triangle within a diagonal 128x128 block: key row kp visible to
    # query col qf iff kp <= qf; duplicated for the 2 heads per tile
    kp = np.arange(128)[:, None]
    qf = np.arange(128)[None, :]
    tri = (kp <= qf).astype(np.float16)
    masks = np.ascontiguousarray(np.concatenate([tri, tri], axis=1))

    in_maps = []
    for c in range(NCORES):
        b = c // 4
        hsel = 4 * (c % 4)
        qt = np.ascontiguousarray(
            q4[b].transpose(1, 2, 0)[hsel:hsel + 4].reshape(2, 128, L)
            .astype(np.float16))
        kt = np.ascontiguousarray(
            k4[b].transpose(1, 2, 0)[hsel:hsel + 4].reshape(2, 128, L)
            .astype(np.float16))
        vsl = v4[b, :, hsel:hsel + 4, :].transpose(1, 0, 2)  # [4, L, DH]
        vext = np.concatenate(
            [vsl, np.ones((PAIRS, L, 1), dtype=np.float32),
             np.zeros((PAIRS, L, 128 - DH - 1), dtype=np.float32)], axis=2)
        # pre-swizzle to the SBUF layout: [pair, partition, chunk*128]
        vext = (vext.reshape(PAIRS, L // KC, KC, 128)
                .transpose(0, 2, 1, 3).reshape(PAIRS, KC, -1))
        vext = np.ascontiguousarray(vext.astype(np.float16))
        wp = np.ascontiguousarray(
            w[(c % 4) * 256:(c % 4 + 1) * 256, :].reshape(2, 128, D)
            .astype(np.float16))
        in_maps.append({"qt": qt, "kt": kt, "v": vext, "masks": masks,
                        "w": wp, "ones": np.ones((1, 64), dtype=np.float16)})
    return in_maps


def kernel(query, key, value, w_proj, b_proj, n_head):
    from concourse.bass_utils import run_bass_kernel_spmd

    bias = np.asarray(b_proj, dtype=np.float32)
    in_maps = _prep_in_maps(query, key, value, w_proj)
    nc = _get_nc()
    res = run_bass_kernel_spmd(nc, in_maps, list(range(NCORES)))

    out = np.zeros((B, L, D), dtype=np.float32)
    for c in range(NCORES):
        out[c // 4] += res.results[c]["out"].astype(np.float32)
    out += bias[None, None, :]
    return out
